# revision 1
# baseline (speedup 1.0000x reference)
"""TOF-weighted PET backprojection + FFT filter (BPF) on 8 Trainium2 NeuronCores.

Algorithm (per core, data-parallel over events):
  - Each of the 250k events deposits Gaussian TOF weights along its
    line-of-response into a 256x256 image (reference samples NS=128 points).
    Only a 32-sample window around the TOF center carries non-negligible
    weight (dropped tail < 3e-6 relative), so we compute exactly those 32
    samples per event with the reference's exact fp32 arithmetic
    (t = (j+0.5)/128 is exact; px = x1 + t*ddx etc.).
  - The scatter-add is performed as dense matmul routing on the PE:
    for each group of 128 samples, build one-hot matrices
    OHYv[k, y] = (iy_k == y) * val_k   (fp16, [128, 256])
    OHX [k, x] = (ix_k == x)           (fp16, [128, 256])
    and accumulate img += OHYv^T @ OHX into PSUM (fp32).
    Out-of-range indices produce all-zero one-hot rows == the reference's
    validity mask.
  - Partial images are AllReduced across the 8 cores, then the (tiny) FFT
    deblur filter is applied on-device as real DFT matmuls:
    out = Re(Fbar @ (G2 * (F @ img @ F)) @ Fbar) * kappa.
"""

import math
import os

import numpy as np

import concourse.bass as bass
import concourse.mybir as mybir
import concourse.tile as tile
from concourse import bacc
from concourse.bass_utils import run_bass_kernel_spmd

F32 = mybir.dt.float32
F16 = mybir.dt.float16
AF = mybir.ActivationFunctionType
OP = mybir.AluOpType

NX = 256
NY = 256
DX = 2.0
TIME_RES = 300.0
EVENT_NUM = 250000
NS = 128
SIGMA = TIME_RES * 0.3 / 2.0 / 2.355   # mm
W = 24                                  # window samples / event
N_CORES = 8
EV_PER_CORE = EVENT_NUM // N_CORES      # 31250
NCOLS_FULL = 256  # 16-col superblocks; 32768 events/core

_FLOOR_MODE = os.environ.get("BPF_FLOOR_MODE", "pymod")  # pymod | shift


def _host_consts():
    """Constant tensors shipped to every core."""
    N = NX
    jk = np.outer(np.arange(N), np.arange(N)) % N
    ang = 2.0 * np.pi * jk / N
    C = np.cos(ang).astype(np.float32)
    S = np.sin(ang).astype(np.float32)
    tof_sigma = TIME_RES * 0.3 / 2.0 / 2.355 / DX
    nx2 = N / 2.0
    x_ = (np.linspace(-nx2 + 0.5, nx2 - 0.5, N) / nx2).astype(np.float32)
    xx, yy = np.meshgrid(x_, x_, indexing="ij")
    w0 = xx * xx + yy * yy
    tmp = w0 * (np.pi * tof_sigma) ** 2
    # i0e via the numerically-stable series/asymptotic split (scipy-free)
    t = np.asarray(tmp, dtype=np.float64)
    small = t < 700.0
    i0e = np.empty_like(t)
    # small: i0e = exp(-x) * I0(x) via np.i0 (safe below overflow)
    i0e[small] = np.exp(-t[small]) * np.i0(t[small])
    # large: asymptotic I0(x)*exp(-x) ~ 1/sqrt(2*pi*x) * (1 + 1/(8x) + 9/(128x^2))
    tl = t[~small]
    i0e[~small] = (1.0 + 1.0 / (8.0 * tl) + 9.0 / (128.0 * tl * tl)) / np.sqrt(
        2.0 * np.pi * tl
    )
    freq_filter = (1.0 / i0e).astype(np.float32)
    G2 = np.fft.ifftshift(freq_filter)
    kappa = N / (N * N) / (DX * DX)
    G2k = (G2 * kappa).astype(np.float32)

    iotaK = np.broadcast_to(
        (np.arange(W, dtype=np.float32) * np.float32(1.0 / NS))[None, :], (128, W)
    ).copy()  # k/128, exact
    iota256 = np.broadcast_to(
        np.arange(256, dtype=np.float16)[None, :], (128, 256)
    ).copy()
    iota_c = np.broadcast_to(
        np.arange(64, 192, dtype=np.float16)[None, :], (128, 128)).copy()
    iota_o = np.broadcast_to(
        np.concatenate([np.arange(0, 64), np.arange(192, 256)]
                       ).astype(np.float16)[None, :], (128, 128)).copy()
    ident16 = np.eye(128, dtype=np.float16)
    ident32 = np.eye(128, dtype=np.float32)
    return {
        "c_dft": C.reshape(2, 128, 256),
        "s_dft": S.reshape(2, 128, 256),
        "negs_dft": (-S).reshape(2, 128, 256),
        "negc_dft": (-C).reshape(2, 128, 256),
        "g2k": G2k.reshape(2, 128, 256),
        "iota_k": iotaK,
        "iota256": iota256,
        "iota_c": iota_c,
        "iota_o": iota_o,
        "ident16": ident16,
        "ident32": ident32,
    }


def build_program(ncols=NCOLS_FULL, n_cores=N_CORES, partial_only=False,
                  active_cols=None, ablate=0, repeat=1):
    """Emit the Bass/Tile program. Events per core = 128 * ncols."""
    assert ncols % 16 == 0
    nsb = (active_cols if active_cols is not None else ncols) // 16
    SBW = 16 * W            # 384 sample slots per row per superblock
    NBLK = SBW // 128       # 3 transposed blocks per superblock
    nc = bacc.Bacc(
        "TRN2",
        target_bir_lowering=False,
        debug=False,
        enable_asserts=False,
        num_devices=n_cores,
    )

    names = [
        "projection_data", "tof_value",
        "x1l", "y1l", "x1r", "y1r", "x2l", "y2l", "x2r", "y2r",
    ]
    ins = {
        n: nc.dram_tensor(n, [128, ncols], F32, kind="ExternalInput") for n in names
    }
    cdef = {
        "c_dft": [2, 128, 256], "s_dft": [2, 128, 256], "negs_dft": [2, 128, 256],
        "negc_dft": [2, 128, 256], "g2k": [2, 128, 256],
        "iota_k": [128, W], "ident32": [128, 128],
    }
    cins = {
        n: nc.dram_tensor(n, shp, F32, kind="ExternalInput")
        for n, shp in cdef.items()
    }
    cins["iota256"] = nc.dram_tensor("iota256", [128, 256], F16, kind="ExternalInput")
    cins["iota_c"] = nc.dram_tensor("iota_c", [128, 128], F16, kind="ExternalInput")
    cins["iota_o"] = nc.dram_tensor("iota_o", [128, 128], F16, kind="ExternalInput")
    cins["ident16"] = nc.dram_tensor("ident16", [128, 128], F16, kind="ExternalInput")
    out_img = nc.dram_tensor("out_img", [2, 128, 256], F32, kind="ExternalOutput")

    exp_scale = float(np.float32(-0.5 / np.float64(SIGMA) ** 2))

    with tile.TileContext(nc) as tc:
        cpool = tc.alloc_tile_pool(name="consts", bufs=1)
        epool = tc.alloc_tile_pool(name="events", bufs=1)
        wpool = tc.alloc_tile_pool(name="work", bufs=2)
        tpool = tc.alloc_tile_pool(name="trans", bufs=2)
        ohpool = tc.alloc_tile_pool(name="oh", bufs=4)
        pimg = tc.alloc_tile_pool(name="pimg", bufs=1, space="PSUM")
        pT = tc.alloc_tile_pool(name="pT", bufs=2, space="PSUM")
        dpool = tc.alloc_tile_pool(name="dram", bufs=1, space="DRAM")

        # ---- constants to SBUF
        csb = {}
        for n in ("iota_k", "ident32"):
            csb[n] = cpool.tile(cdef[n], F32, tag=n, name=n + "_sb")
            nc.sync.dma_start(out=csb[n][:], in_=cins[n][:])
        for n in ("iota256", "iota_c", "iota_o", "ident16"):
            shp = [128, 256] if n == "iota256" else [128, 128]
            csb[n] = cpool.tile(shp, F16, tag=n, name=n + "_sb")
            nc.sync.dma_start(out=csb[n][:], in_=cins[n][:])
        for n in ("c_dft", "s_dft", "negs_dft", "negc_dft", "g2k"):
            csb[n] = [cpool.tile([128, 256], F32, tag=f"{n}{h}", name=f"{n}{h}_sb") for h in range(2)]
            for h in range(2):
                nc.sync.dma_start(out=csb[n][h][:], in_=cins[n][h])

        # ---- event inputs to SBUF
        esb = {}
        for n in names:
            esb[n] = epool.tile([128, ncols], F32, tag=n, name=n + "_esb")
            nc.sync.dma_start(out=esb[n][:], in_=ins[n][:])

        def ev(tag):
            return epool.tile([128, ncols], F32, tag=tag, name=tag)

        V = nc.vector

        # ---- phase 1: per-event scalars (order matches reference fp32 rounding)
        x1 = ev("x1"); y1 = ev("y1"); ddx = ev("ddx"); ddy = ev("ddy")
        V.tensor_tensor(out=x1[:], in0=esb["x1l"][:], in1=esb["x1r"][:], op=OP.add)
        V.tensor_scalar(out=x1[:], in0=x1[:], scalar1=0.5, scalar2=None, op0=OP.mult)
        V.tensor_tensor(out=y1[:], in0=esb["y1l"][:], in1=esb["y1r"][:], op=OP.add)
        V.tensor_scalar(out=y1[:], in0=y1[:], scalar1=0.5, scalar2=None, op0=OP.mult)
        x2 = ev("x2"); y2 = ev("y2")
        V.tensor_tensor(out=x2[:], in0=esb["x2l"][:], in1=esb["x2r"][:], op=OP.add)
        V.tensor_scalar(out=x2[:], in0=x2[:], scalar1=0.5, scalar2=None, op0=OP.mult)
        V.tensor_tensor(out=y2[:], in0=esb["y2l"][:], in1=esb["y2r"][:], op=OP.add)
        V.tensor_scalar(out=y2[:], in0=y2[:], scalar1=0.5, scalar2=None, op0=OP.mult)
        V.tensor_tensor(out=ddx[:], in0=x2[:], in1=x1[:], op=OP.subtract)
        V.tensor_tensor(out=ddy[:], in0=y2[:], in1=y1[:], op=OP.subtract)

        L = ev("L"); tmp = ev("tmp"); tmp2 = ev("tmp2")
        V.tensor_tensor(out=tmp[:], in0=ddx[:], in1=ddx[:], op=OP.mult)
        V.tensor_tensor(out=tmp2[:], in0=ddy[:], in1=ddy[:], op=OP.mult)
        V.tensor_tensor(out=tmp[:], in0=tmp[:], in1=tmp2[:], op=OP.add)
        nc.scalar.activation(out=L[:], in_=tmp[:], func=AF.Sqrt)
        invL = ev("invL")
        V.reciprocal(out=invL[:], in_=L[:])

        center = ev("center")
        V.tensor_scalar(out=tmp[:], in0=L[:], scalar1=0.5, scalar2=None, op0=OP.mult)
        V.scalar_tensor_tensor(
            out=center[:], in0=esb["tof_value"][:], scalar=0.15, in1=tmp[:],
            op0=OP.mult, op1=OP.add)

        # jc = center*invL*128 - 0.5 ; j0 = clip(floor(jc)-(W/2-1), 0, 128-W)
        jc = ev("jc")
        V.tensor_tensor(out=jc[:], in0=center[:], in1=invL[:], op=OP.mult)
        V.tensor_scalar(out=jc[:], in0=jc[:], scalar1=128.0, scalar2=0.5,
                        op0=OP.mult, op1=OP.subtract)
        jci = epool.tile([128, ncols], mybir.dt.int32, tag="jci", name="jci")
        V.tensor_scalar(out=jci[:], in0=jc[:], scalar1=0.5, scalar2=None,
                        op0=OP.subtract)   # rne(jc-0.5) == floor for non-int jc
        V.tensor_copy(out=jc[:], in_=jci[:])
        V.tensor_scalar(out=jc[:], in0=jc[:], scalar1=float(W // 2 - 1),
                        scalar2=0.0, op0=OP.subtract, op1=OP.max)
        base_t = ev("base_t")
        V.tensor_scalar(out=jc[:], in0=jc[:], scalar1=float(128 - W), scalar2=0.5,
                        op0=OP.min, op1=OP.add)
        V.tensor_scalar(out=base_t[:], in0=jc[:], scalar1=1.0 / 128.0, scalar2=None,
                        op0=OP.mult)   # (j0+0.5)/128, exact

        amp = ev("amp")
        V.tensor_scalar(out=tmp[:], in0=L[:], scalar1=1.0 / 128.0, scalar2=None,
                        op0=OP.mult)
        V.tensor_tensor(out=amp[:], in0=esb["projection_data"][:], in1=tmp[:],
                        op=OP.mult)

        # ---- phase 2+3: superblocks of 16 event-cols
        ps_main = pimg.tile([128, 256], F32, tag="ps_main", name="ps_main")
        ps_rest = pimg.tile([128, 256], F32, tag="ps_rest", name="ps_rest")
        n_groups = nsb * SBW * repeat
        n_rest = SBW * repeat   # last superblock per pass is CC
        gi = 0
        ri = 0

        iota_k3 = csb["iota_k"][:, None, :].to_broadcast([128, 16, W])

        for q_rep in range(nsb * repeat):
            q = q_rep % nsb
            evs = slice(16 * q, 16 * (q + 1))

            def eb(t):
                return t[:, evs][:, :, None].to_broadcast([128, 16, W])

            T4 = wpool.tile([128, 16, W], F32, tag="T4", name="T4")
            U4 = wpool.tile([128, 16, W], F32, tag="U4", name="U4")
            V4 = wpool.tile([128, 16, W], F32, tag="V4", name="V4")
            Z4 = wpool.tile([128, 16, W], F32, tag="Z4", name="Z4")
            WW = wpool.tile([128, 16, W], F32, tag="WW", name="WW")
            IXh = wpool.tile([128, 16, W], F32, tag="IXh", name="IXh")
            IYh = wpool.tile([128, 16, W], F32, tag="IYh", name="IYh")
            VALh = wpool.tile([128, 16, W], F32, tag="VALh", name="VALh")
            V.tensor_tensor(out=T4[:], in0=iota_k3, in1=eb(base_t), op=OP.add)
            V.tensor_tensor(out=U4[:], in0=T4[:], in1=eb(ddx), op=OP.mult)
            V.tensor_tensor(out=U4[:], in0=U4[:], in1=eb(x1), op=OP.add)
            V.tensor_scalar(out=U4[:], in0=U4[:], scalar1=0.5, scalar2=128.0,
                            op0=OP.mult, op1=OP.add)
            V.tensor_tensor(out=V4[:], in0=T4[:], in1=eb(ddy), op=OP.mult)
            V.tensor_tensor(out=V4[:], in0=V4[:], in1=eb(y1), op=OP.add)
            V.tensor_scalar(out=V4[:], in0=V4[:], scalar1=0.5, scalar2=128.0,
                            op0=OP.mult, op1=OP.add)
            V.tensor_tensor(out=Z4[:], in0=T4[:], in1=eb(L), op=OP.mult)
            V.tensor_tensor(out=Z4[:], in0=Z4[:], in1=eb(center), op=OP.subtract)
            V.tensor_tensor(out=Z4[:], in0=Z4[:], in1=Z4[:], op=OP.mult)
            nc.scalar.activation(out=WW[:], in_=Z4[:], func=AF.Exp, scale=exp_scale)
            V.tensor_tensor(out=VALh[:], in0=WW[:], in1=eb(amp), op=OP.mult)
            # exact floor: R = rne(u); floor = R - (R > u)
            IXi = wpool.tile([128, 16, W], mybir.dt.int32, tag="IXi", name="IXi")
            RR = wpool.tile([128, 16, W], F32, tag="RR", name="RR")
            GG = wpool.tile([128, 16, W], F32, tag="GG", name="GG")
            V.tensor_copy(out=IXi[:], in_=U4[:])
            V.tensor_copy(out=RR[:], in_=IXi[:])
            V.tensor_tensor(out=GG[:], in0=RR[:], in1=U4[:], op=OP.is_gt)
            V.tensor_tensor(out=IXh[:], in0=RR[:], in1=GG[:], op=OP.subtract)
            V.tensor_copy(out=IXi[:], in_=V4[:])
            V.tensor_copy(out=RR[:], in_=IXi[:])
            V.tensor_tensor(out=GG[:], in0=RR[:], in1=V4[:], op=OP.is_gt)
            V.tensor_tensor(out=IYh[:], in0=RR[:], in1=GG[:], op=OP.subtract)

            # transpose to [sample-slot, row] layout, 3 blocks each
            IXT = tpool.tile([128, SBW], F32, tag="IXT", name="IXT")
            IYT = tpool.tile([128, SBW], F32, tag="IYT", name="IYT")
            VALT = tpool.tile([128, SBW], F32, tag="VALT", name="VALT")
            for src, dst, tg in ((IXh, IXT, "px"), (IYh, IYT, "py"),
                                 (VALh, VALT, "pv")):
                flat = src[:].rearrange("p a b -> p (a b)")
                for j in range(NBLK):
                    ps = pT.tile([128, 128], F32, tag="tps", name=tg, bufs=4)
                    nc.tensor.transpose(out=ps[:], in_=flat[:, 128*j:128*(j+1)],
                                        identity=csb["ident32"][:])
                    nc.scalar.copy(out=dst[:, 128*j:128*(j+1)], in_=ps[:])

            is_cc = (q == nsb - 1)
            for j in range(NBLK):
                for g in range(128):
                    col = slice(128*j + g, 128*j + g + 1)
                    first = gi == 0
                    last = gi == n_groups - 1
                    if not is_cc:
                        ohx = ohpool.tile([128, 128], F16, tag="ohxc", name="ohx")
                        ohyv = ohpool.tile([128, 128], F16, tag="ohyc", name="ohyv")
                        if ablate < 2:
                            V.tensor_scalar(out=ohx[:], in0=csb["iota_c"][:],
                                            scalar1=IXT[:, col], scalar2=None,
                                            op0=OP.is_equal)
                            V.tensor_scalar(out=ohyv[:], in0=csb["iota_c"][:],
                                            scalar1=IYT[:, col], scalar2=VALT[:, col],
                                            op0=OP.is_equal, op1=OP.mult)
                        if ablate < 1:
                            nc.tensor.matmul(ps_main[:, 64:192], lhsT=ohyv[:],
                                             rhs=ohx[:], start=first, stop=last,
                                             skip_group_check=True)
                    else:
                        ohx = ohpool.tile([128, 256], F16, tag="ohx", name="ohx")
                        ohyv = ohpool.tile([128, 128], F16, tag="ohyc", name="ohyv")
                        ohyo = ohpool.tile([128, 128], F16, tag="ohyo", name="ohyo")
                        if ablate < 2:
                            V.tensor_scalar(out=ohx[:], in0=csb["iota256"][:],
                                            scalar1=IXT[:, col], scalar2=None,
                                            op0=OP.is_equal)
                            V.tensor_scalar(out=ohyv[:], in0=csb["iota_c"][:],
                                            scalar1=IYT[:, col], scalar2=VALT[:, col],
                                            op0=OP.is_equal, op1=OP.mult)
                            V.tensor_scalar(out=ohyo[:], in0=csb["iota_o"][:],
                                            scalar1=IYT[:, col], scalar2=VALT[:, col],
                                            op0=OP.is_equal, op1=OP.mult)
                        if ablate < 1:
                            nc.tensor.matmul(ps_main[:], lhsT=ohyv[:],
                                             rhs=ohx[:], start=first, stop=last,
                                             skip_group_check=True)
                            nc.tensor.matmul(ps_rest[:], lhsT=ohyo[:],
                                             rhs=ohx[:], start=(ri == 0),
                                             stop=(ri == n_rest - 1),
                                             skip_group_check=True)
                        ri += 1
                    gi += 1

        # ---- phase 4: partial image -> DRAM, AllReduce
        img_sb = wpool.tile([128, 512], F32, tag="img_sb", name="img_sb")
        if ablate < 1:
            # ps_main partitions 0..127 = rows 64..191; ps_rest: p<64 -> row p,
            # p>=64 -> row 128+p. img_sb: [:, 0:256]=rows 0..127, [:,256:512]=128..255
            V.tensor_copy(out=img_sb[64:128, 0:256], in_=ps_main[0:64, :])
            V.tensor_copy(out=img_sb[0:64, 256:512], in_=ps_main[64:128, :])
            V.tensor_copy(out=img_sb[0:64, 0:256], in_=ps_rest[0:64, :])
            V.tensor_copy(out=img_sb[64:128, 256:512], in_=ps_rest[64:128, :])
        else:
            nc.vector.memset(img_sb[:], 0)
        bounce = dpool.tile([2, 128, 256], F32, tag="bounce", name="bounce")
        bounce2 = dpool.tile([2, 128, 256], F32, tag="bounce2", name="bounce2")
        for h in range(2):
            nc.sync.dma_start(out=bounce[h], in_=img_sb[:, 256 * h:256 * (h + 1)])
        if not partial_only:
            nc.gpsimd.collective_compute(
                "AllReduce", OP.add,
                replica_groups=[list(range(n_cores))],
                ins=[bounce.opt()], outs=[bounce2.opt()],
            )
        else:
            nc.sync.dma_start(out=bounce2[0], in_=img_sb[:, 0:256])
            nc.sync.dma_start(out=bounce2[1], in_=img_sb[:, 256:512])

        # ---- phase 5: DFT filter. B = allreduced img [2][128,256]
        B = [tpool.tile([128, 256], F32, tag=f"B{h}", name=f"B{h}") for h in range(2)]
        for h in range(2):
            nc.sync.dma_start(out=B[h][:], in_=bounce2[h])

        Cd, Sd, nSd, nCd = csb["c_dft"], csb["s_dft"], csb["negs_dft"], csb["negc_dft"]

        def product_pair(outs, terms, tag):
            res = []
            for mh in range(2):
                ps = pT.tile([128, 256], F32, tag="prodps", name=f"{tag}ps", bufs=2)
                n_mm = len(terms) * 2
                i = 0
                for lconst, rhs in terms:
                    for kh in range(2):
                        nc.tensor.matmul(
                            ps[:], lhsT=lconst[kh][:, 128 * mh:128 * (mh + 1)],
                            rhs=rhs[kh][:], start=(i == 0), stop=(i == n_mm - 1),
                            skip_group_check=True)
                        i += 1
                sb = tpool.tile([128, 256], F32, tag=f"{tag}{mh}", name=f"{tag}{mh}")
                V.tensor_copy(out=sb[:], in_=ps[:])
                res.append(sb)
            return res

        def transpose256(M, tag):
            res = [tpool.tile([128, 256], F32, tag=f"{tag}{h}", name=f"{tag}{h}") for h in range(2)]
            for a in range(2):
                for bb in range(2):
                    ps = pT.tile([128, 128], F32, tag="tps", name=f"{tag}ps", bufs=4)
                    nc.tensor.transpose(
                        out=ps[:], in_=M[bb][:, 128 * a:128 * (a + 1)],
                        identity=csb["ident32"][:])
                    V.tensor_copy(out=res[a][:, 128 * bb:128 * (bb + 1)], in_=ps[:])
            return res

        Pr = product_pair(None, [(Cd, B)], "Pr")          # C @ B
        Pi = product_pair(None, [(nSd, B)], "Pi")         # -S @ B
        PrT = transpose256(Pr, "PrT")
        PiT = transpose256(Pi, "PiT")
        QrT = product_pair(None, [(Cd, PrT), (Sd, PiT)], "Qr")    # C@PrT + S@PiT
        QiT = product_pair(None, [(Cd, PiT), (nSd, PrT)], "Qi")   # C@PiT - S@PrT
        HrT = [tpool.tile([128, 256], F32, tag=f"Hr{h}", name=f"Hr{h}") for h in range(2)]
        HiT = [tpool.tile([128, 256], F32, tag=f"Hi{h}", name=f"Hi{h}") for h in range(2)]
        for h in range(2):
            V.tensor_tensor(out=HrT[h][:], in0=QrT[h][:], in1=csb["g2k"][h][:],
                            op=OP.mult)
            V.tensor_tensor(out=HiT[h][:], in0=QiT[h][:], in1=csb["g2k"][h][:],
                            op=OP.mult)
        TrT = product_pair(None, [(Cd, HrT), (nSd, HiT)], "Tr")   # C@HrT - S@HiT
        TiT = product_pair(None, [(Cd, HiT), (Sd, HrT)], "Ti")    # C@HiT + S@HrT
        Tr = transpose256(TrT, "TrU")
        Ti = transpose256(TiT, "TiU")
        Out = product_pair(None, [(Cd, Tr), (nSd, Ti)], "Out")    # C@Tr - S@Ti
        for h in range(2):
            nc.sync.dma_start(out=out_img[h], in_=Out[h][:])

        for p in (dpool, pT, pimg, ohpool, tpool, wpool, epool, cpool):
            p.release()

    nc.compile()
    return nc


def _pad_shard(inputs, ncols=NCOLS_FULL, n_cores=N_CORES):
    """Split events across cores; sort AA (center-window) events into the
    first nsb-1 superblocks, everything else + padding into the last."""
    E = inputs["projection_data"].shape[0]
    per = int(math.ceil(E / n_cores))
    cap = 128 * ncols
    assert per <= cap
    nsb = ncols // 16
    aa_cap = 128 * 16 * (nsb - 1)
    pad_vals = {
        "projection_data": 0.0, "tof_value": 0.0,
        "x1l": 400.0, "y1l": 0.0, "x1r": 400.0, "y1r": 0.0,
        "x2l": -400.0, "y2l": 0.0, "x2r": -400.0, "y2r": 0.0,
    }
    arrs = {k: np.asarray(v, dtype=np.float32) for k, v in inputs.items()}
    # conservative window classification (mirrors device arithmetic +- slack)
    x1 = 0.5 * (arrs["x1l"] + arrs["x1r"]); y1 = 0.5 * (arrs["y1l"] + arrs["y1r"])
    x2 = 0.5 * (arrs["x2l"] + arrs["x2r"]); y2 = 0.5 * (arrs["y2l"] + arrs["y2r"])
    ddx = (x2 - x1).astype(np.float32); ddy = (y2 - y1).astype(np.float32)
    with np.errstate(divide="ignore", invalid="ignore"):
        L = np.sqrt(ddx * ddx + ddy * ddy).astype(np.float32)
        center = (0.5 * L + arrs["tof_value"] * np.float32(0.15)).astype(np.float32)
        jc = center * (1.0 / np.maximum(L, 1e-20)) * np.float32(128.0) - np.float32(0.5)
    jc = np.nan_to_num(jc, nan=64.0, posinf=64.0, neginf=64.0)
    j0 = np.clip(np.floor(jc) - (W // 2 - 1), 0.0, 128.0 - W).astype(np.float32)
    lo, hi = [], []
    for jj in (j0 - 1.0, j0 + W):
        t = ((jj + 0.5) * np.float32(1.0 / 128.0)).astype(np.float32)
        u = np.floor((x1 + t * ddx) * 0.5 + 128.0)
        v = np.floor((y1 + t * ddy) * 0.5 + 128.0)
        lo.append((u, v)); hi.append((u, v))
    ul = np.minimum(lo[0][0], lo[1][0]) - 2; uh = np.maximum(lo[0][0], lo[1][0]) + 2
    vl = np.minimum(lo[0][1], lo[1][1]) - 2; vh = np.maximum(lo[0][1], lo[1][1]) + 2
    aa = (vl >= 64) & (vh <= 191) & (ul >= 64) & (uh <= 191)

    shards = []
    for c in range(n_cores):
        s0, s1 = c * per, min((c + 1) * per, E)
        idx = np.arange(s0, s1)
        aa_idx = idx[aa[s0:s1]]
        cc_idx = idx[~aa[s0:s1]]
        if len(aa_idx) > aa_cap:
            cc_idx = np.concatenate([cc_idx, aa_idx[aa_cap:]])
            aa_idx = aa_idx[:aa_cap]
        n_pad = cap - len(aa_idx) - len(cc_idx)
        assert n_pad >= 0, "CC superblock overflow: too many straddling events"
        aa_part = np.concatenate([aa_idx, np.full(aa_cap - len(aa_idx), -1)])
        cc_part = np.concatenate([cc_idx, np.full(cap - aa_cap - len(cc_idx), -1)])
        order2d = np.empty((128, ncols), dtype=np.int64)
        order2d[:, : 16 * (nsb - 1)] = aa_part.reshape(128, 16 * (nsb - 1))
        order2d[:, 16 * (nsb - 1):] = cc_part.reshape(128, 16)
        m = {}
        for k, v in arrs.items():
            a = np.full((128, ncols), pad_vals[k], dtype=np.float32)
            sel = order2d >= 0
            a[sel] = v[order2d[sel]]
            m[k] = a
        shards.append(m)
    return shards


_CACHE = {}


def _get_program(ncols, n_cores):
    key = (ncols, n_cores)
    if key not in _CACHE:
        _CACHE[key] = build_program(ncols, n_cores)
    return _CACHE[key]


def run(inputs, ncols=NCOLS_FULL, n_cores=N_CORES, trace=False):
    nc = _get_program(ncols, n_cores)
    consts = _host_consts()
    shards = _pad_shard(inputs, ncols, n_cores)
    in_maps = [dict(s, **consts) for s in shards]
    res = run_bass_kernel_spmd(nc, in_maps, list(range(n_cores)), trace=trace)
    out = res.results[0]["out_img"].reshape(256, 256)
    return out.astype(np.float32), res


def kernel(**inputs):
    out, _ = run(inputs)
    return out

